# revision 21
# baseline (speedup 1.0000x reference)
"""Multi-head attention (B=2, S=2048, D=1024, 16 heads x 64) on 8 NeuronCores.

Sharding: batch x head-group data/tensor parallel. Core c handles batch
c//4 and heads [4*(c%4), 4*(c%4)+4) as two head-PAIRS. Wq/Wk/Wv are
column-sliced per head group, Wo row-sliced; each core emits a partial
[S, D] fp16 output and the host sums the 4 partials per batch and adds
bo_eff = bo + bv @ Wo (the V-bias commutes through softmax-weighted
averaging, and bk cancels inside the softmax entirely, so neither is
computed on-device).

Matmul data is fp16 (fp32r is 3x slower, fp8 fails the error budget).
Accumulation is fp32 in PSUM.

Per-core kernel (software-pipelined single stream):
  1. K^T, Q^T projections in transposed layout [inner, seq], both
     k-major chasing their per-chunk input DMAs (xk/xq are 8 separate
     [128, S] tiles so the first matmul only waits for chunk 0, not the
     whole 4MB stream). bq is applied on the PSUM->SBUF move.
  2. V projection in NATURAL layout [seq, dh] (stationary = X_v^T chunk,
     moving = Wv rows): no PE transposes needed. PSUM->SBUF copies (on
     the otherwise-idle gpsimd engine) place each head's V as
     [dh(64) | 1] per 128-key chunk (ones column => attn@V row 64 =
     softmax denominator).
  3. Attention per (q-slab s of 512, pair p, key-chunk j):
     - scores: TWO row-tiled matmuls run CONCURRENTLY on the PE array
       (head A on array rows 0-63, head B on rows 64-127 via
       tile_position) -- 2 heads in the time of one. Emitted LAST in
       each slot so their wait on the exp(i-2) PSUM-release overlaps
       the slot's attnV matmuls instead of stalling the PE.
     - exp: 3 of 4 chunks on ACT (exact, scale=1/8 folded); every 4th
       chunk on DVE via the fp16 Schraudolph bit-trick
       (i16 = s*184.665 + 15301.13; bitcast -> ~exp(s/8), ~1.8% rms)
       to relieve the ACT engine, which is otherwise the bottleneck.
     - attn@V: per head [V|1].T @ exp -> av[0:65] PSUM accumulated over
       16 chunks; row 64 = denominator.
     - normalize: one reciprocal per (p, s) on the joint [1, 1024]
       denominator row, gpsimd partition-broadcast, DVE multiplies
       into O^T.
     V-projection tiles and output-projection tiles drip into the
     stream as PE filler.
  4. Output projection per 128-row chunk t: both D-halves under one
     stationary load per contraction step (halves inner, c outer), fp16
     out. The last slab's normalize runs at 128-column granularity so
     its fo chunks start as soon as their columns are normalized,
     shortening the tail's serial norm->fo chain.
"""

import sys

if "/opt/trn_rl_repo" not in sys.path:
    sys.path.insert(0, "/opt/trn_rl_repo")

import numpy as np

import concourse.bacc as bacc
import concourse.mybir as mybir
from concourse.bass_utils import run_bass_kernel_spmd

F32 = mybir.dt.float32
F16 = mybir.dt.float16
I16 = mybir.dt.int16
NPDT = np.float16

B, S, D = 2, 2048, 1024
NH, DH = 16, 64
NCORES = 8
GROUPS = 4                # head-groups (cores per batch)
HG = NH // GROUPS         # heads per core = 4
NP = HG // 2              # head-pairs per core = 2
IS = HG * DH              # inner slice per core = 256
KD = D // 128             # contraction chunks for projections = 8
KT = S // 128             # 128-row key chunks = 16
W = 512                   # q sub-slab width
NS = S // W               # sub-slabs = 4

# Schraudolph fp16 exp(x/8) constants (C tuned for zero-mean rel err)
SCH_A = 0.125 * 1024.0 / float(np.log(2.0))   # 184.665
SCH_B = 15360.0 - 58.87
DVE_EXP = True

_CACHE = {}


def _build_nc():
    nc = bacc.Bacc("TRN2", target_bir_lowering=False, debug=False)

    xqT = nc.dram_tensor("xqT", [D, S], F16, kind="ExternalInput").ap()
    xkT = nc.dram_tensor("xkT", [D, S], F16, kind="ExternalInput").ap()
    xvT = nc.dram_tensor("xvT", [D, S], F16, kind="ExternalInput").ap()
    wq = nc.dram_tensor("wq", [D, IS], F16, kind="ExternalInput").ap()
    wk = nc.dram_tensor("wk", [D, IS], F16, kind="ExternalInput").ap()
    wv = nc.dram_tensor("wv", [D, IS], F16, kind="ExternalInput").ap()
    wo = nc.dram_tensor("wo", [IS, D], F16, kind="ExternalInput").ap()
    bq = nc.dram_tensor("bq", [IS], F32, kind="ExternalInput").ap()
    out = nc.dram_tensor("out", [S, D], F16, kind="ExternalOutput").ap()

    with __import__("concourse.tile", fromlist=["TileContext"]).TileContext(nc) as tc:
        _emit(nc, tc, xqT, xkT, xvT, wq, wk, wv, wo, bq, out)
    nc.compile()
    return nc


def _emit(nc, tc, xqT, xkT, xvT, wq, wk, wv, wo, bq, out):
    from contextlib import ExitStack

    ctx = ExitStack()
    with ctx:
        consts = ctx.enter_context(tc.tile_pool(name="consts", bufs=1))
        big = ctx.enter_context(tc.tile_pool(name="big", bufs=1))
        # exp runway: scores/exp run ~LAG tiles ahead of the attnV stream
        # (opened after the scoped xkq pool below frees its 64KB)
        smallp = ctx.enter_context(tc.tile_pool(name="smallp", bufs=2))
        outp = ctx.enter_context(tc.tile_pool(name="outp", bufs=2))

        wk_sb = consts.tile([128, KD, IS], F16, name="wk_sb")
        wq_sb = consts.tile([128, KD, IS], F16, name="wq_sb")
        wv_sb = consts.tile([128, KD, IS], F16, name="wv_sb")
        wo_sb = consts.tile([128, NP, D], F16, name="wo_sb")
        bq_sb = consts.tile([128, NP], F32, name="bq_sb")

        # persistent intermediates
        KT_sb = big.tile([128, NP, S], F16, name="KT_sb")   # pair-packed K^T
        QT_sb = big.tile([128, NP, S], F16, name="QT_sb")
        xv_sb = big.tile([128, KD, S], F16, name="xv_sb")
        # V chunks: per (pair, key-chunk, head-in-pair): [dh(64) | ones]
        V_sb = big.tile([128, NP, KT, 2, DH + 1], F16, name="V_sb")
        OT_sb = big.tile([128, NP, S], F16, name="OT_sb")

        # engine alternator for PSUM->SBUF moves
        _eng = [0]

        def alt_copy(dst_ap, src_ap, bias_ap=None):
            if _eng[0] % 2 == 0:
                if bias_ap is None:
                    nc.vector.tensor_copy(dst_ap, src_ap)
                else:
                    nc.vector.tensor_scalar_add(dst_ap, src_ap, bias_ap)
            else:
                if bias_ap is None:
                    nc.scalar.copy(dst_ap, src_ap)
                else:
                    nc.scalar.activation(
                        dst_ap, src_ap,
                        mybir.ActivationFunctionType.Identity,
                        bias=bias_ap,
                    )
            _eng[0] += 1

        # ---- stage 1: K^T and Q^T projections, k-major DMA-chasing ----
        NS2 = S // 512
        with nc.named_scope("proj"):
            with tc.tile_pool(name="psP", bufs=8, space="PSUM") as psP, \
                 tc.tile_pool(name="xkq", bufs=1) as xkq:
                # raw X^T chunks: one tile per 128-row contraction chunk
                # so the consuming matmuls gate on the per-chunk DMA, not
                # the full stream; the scoped pool frees their 64KB for
                # the attention phase's exp runway
                xk_sb = [
                    xkq.tile([128, S], F16, name=f"xk_sb{k}")
                    for k in range(KD)
                ]
                xq_sb = [
                    xkq.tile([128, S], F16, name=f"xq_sb{k}")
                    for k in range(KD)
                ]
                # wk/wq ride the scalar engine's HWDGE queue; the less
                # urgent bq/wv/wo go on the idle gpsimd queue so neither
                # the big sync-queue input stream nor the scalar queue's
                # upcoming copies are stalled by small strided transfers
                nc.scalar.dma_start(
                    out=wk_sb, in_=wk.rearrange("(k p) i -> p k i", p=128)
                )
                nc.scalar.dma_start(
                    out=wq_sb, in_=wq.rearrange("(k p) i -> p k i", p=128)
                )
                nc.gpsimd.dma_start(
                    out=bq_sb, in_=bq.rearrange("(m p) -> p m", p=128)
                )
                # pure streaming input DMAs: xk, xq, xv sequential on the
                # sync queue, no SBUF flow control (persistent dest tiles);
                # wv/wo ride the same queue AFTER xv so their ~1MB doesn't
                # steal HBM wire from the critical xk/xq stream
                for k in range(KD):
                    nc.sync.dma_start(
                        out=xk_sb[k], in_=xkT[128 * k:128 * (k + 1), :]
                    )
                for k in range(KD):
                    nc.sync.dma_start(
                        out=xq_sb[k], in_=xqT[128 * k:128 * (k + 1), :]
                    )
                nc.sync.dma_start(
                    out=xv_sb, in_=xvT.rearrange("(k p) s -> p k s", p=128)
                )
                nc.sync.dma_start(
                    out=wv_sb, in_=wv.rearrange("(k p) i -> p k i", p=128)
                )
                nc.sync.dma_start(
                    out=wo_sb, in_=wo.rearrange("(c p) d -> p c d", p=128)
                )
                # K projection, k-major (streams behind the xk DMAs)
                ps = [
                    [
                        psP.tile([128, 512], F32, tag="ps", name=f"ps_{m}_{n}")
                        for n in range(NS2)
                    ]
                    for m in range(NP)
                ]
                for k in range(KD):
                    if k == 0:
                        # HAM warm-up: throwaway pass while DMAs stream
                        for m in range(NP):
                            for n in range(NS2):
                                nc.tensor.matmul(
                                    ps[m][n],
                                    wk_sb[:, 0, 128 * m:128 * (m + 1)],
                                    xk_sb[0][:, 512 * n:512 * (n + 1)],
                                    start=True, stop=True)
                    for m in range(NP):
                        for n in range(NS2):
                            nc.tensor.matmul(
                                ps[m][n],
                                wk_sb[:, k, 128 * m:128 * (m + 1)],
                                xk_sb[k][:, 512 * n:512 * (n + 1)],
                                start=(k == 0),
                                stop=(k == KD - 1),
                            )
                for m in range(NP):
                    for n in range(NS2):
                        alt_copy(
                            KT_sb[:, m, 512 * n:512 * (n + 1)],
                            ps[m][n],
                        )
                # Q projection, also k-major: chases the xq DMA chunk by
                # chunk, so it finishes ~DMA-bound instead of waiting for
                # the full xq stream before the first matmul
                qp = [
                    [
                        psP.tile([128, 512], F32, tag="ps", name=f"qp_{m}_{n}")
                        for n in range(NS2)
                    ]
                    for m in range(NP)
                ]
                for k in range(KD):
                    for m in range(NP):
                        for n in range(NS2):
                            nc.tensor.matmul(
                                qp[m][n],
                                wq_sb[:, k, 128 * m:128 * (m + 1)],
                                xq_sb[k][:, 512 * n:512 * (n + 1)],
                                start=(k == 0),
                                stop=(k == KD - 1),
                            )
                # copies ordered by the attention stream's needs (slab 0
                # first, both pairs)
                for n in range(NS2):
                    for m in range(NP):
                        alt_copy(
                            QT_sb[:, m, 512 * n:512 * (n + 1)],
                            qp[m][n],
                            bq_sb[:, m:m + 1],
                        )

        # ---- stage 2: attention + V-proj drip + out-proj drip ----
        # exp runway: scores/exp run ~LAG tiles ahead of the attnV stream
        expp = ctx.enter_context(tc.tile_pool(name="expp", bufs=24))
        avP = ctx.enter_context(tc.tile_pool(name="avP", bufs=1, space="PSUM"))
        spP = ctx.enter_context(tc.tile_pool(name="spP", bufs=2, space="PSUM"))

        # ones columns of every V chunk
        nc.vector.memset(V_sb[:, :, :, :, DH:DH + 1], 1.0)

        def emit_vproj(t):
            vp = spP.tile([128, 512], F32, tag="sp", name="vp")
            for k in range(KD):
                nc.tensor.matmul(
                    vp[:, 0:IS],
                    xv_sb[:, k, 128 * t:128 * (t + 1)],
                    wv_sb[:, k, :],
                    start=(k == 0),
                    stop=(k == KD - 1),
                )
            vph = vp.rearrange("p (g d) -> p g d", g=8, d=DH)
            for p in range(NP):
                alt_copy(
                    V_sb[:, p, t, :, 0:DH],
                    vph[:, 2 * p:2 * p + 2, :],
                )

        def emit_fo(t, pool, tag, tail=False):
            # both D-halves of chunk t: halves inner so each stationary
            # load (OT chunk c) serves two matmuls
            fos = [
                pool.tile([128, 512], F32, tag=tag, name=f"fo{h}")
                for h in range(2)
            ]
            for c in range(NP):
                for half in range(2):
                    nc.tensor.matmul(
                        fos[half],
                        OT_sb[:, c, 128 * t:128 * (t + 1)],
                        wo_sb[:, c, 512 * half:512 * (half + 1)],
                        start=(c == 0),
                        stop=(c == NP - 1),
                    )
            ob = outp.tile([128, D], F16, tag="ob", name="ob")
            for half in range(2):
                dst = ob[:, 512 * half:512 * (half + 1)]
                if tail:
                    # ACT is exp-free in the tail; DVE runs the norm muls
                    nc.scalar.activation(
                        dst, fos[half], mybir.ActivationFunctionType.Copy
                    )
                else:
                    # keep ACT clear for the exp stream during slots
                    nc.vector.tensor_copy(dst, fos[half])
            nc.sync.dma_start(out=out[128 * t:128 * (t + 1), :], in_=ob)

        # ---- two coupled streams with a lag ----
        # Engine queues are strict FIFO: every instruction must be emitted
        # at a queue position matching its execution time, or it blocks
        # its whole engine. The scores/exp stream runs first; the attnV
        # stream trails by LAG slots so that by the time the PE FIFO
        # reaches attnV chunk j, the V projection (gated on the late xv
        # DMA) has produced V(j). V-proj tasks are emitted just-in-time
        # 2 slots before their first consumer.
        LAG = 12
        SLOTS = NS * NP * KT
        fq = []                       # out-proj task queue (t indices)
        normq = []                    # dripped norm stages (one op per slot)
        avs = {}
        exs = {}

        def group(i):
            g, j = divmod(i, KT)
            s, p = divmod(g, NP)
            return g, s, p, j

        def queue_norm(osrc, zsrc, zr, sa, pa, fine):
            """Split the normalize chain into single-op stages so no engine
            queue ever has a multi-microsecond norm chain at its head. The
            rec/bcs stages are halved (per head) so the chain pipelines
            across DVE -> gpsimd -> DVE. For the final group, osrc/zsrc
            point straight at the av PSUM (no oz copy needed since av is
            never reused) and the muls run at 128-column granularity,
            releasing each out-proj chunk as soon as its columns are
            normalized."""
            st = {}

            def mk_rec(hh):
                def s():
                    r = smallp.tile([1, W], F32, tag=f"rec{hh}", name="rec")
                    st[f"rec{hh}"] = r
                    nc.vector.reciprocal_approx_fast(
                        r, zsrc[zr:zr + 1, W * hh:W * (hh + 1)]
                    )
                return s

            def mk_bcs(hh):
                def s():
                    b = smallp.tile([DH, W], F32, tag=f"bcs{hh}", name="bcs")
                    st[f"bcs{hh}"] = b
                    nc.gpsimd.partition_broadcast(b, st[f"rec{hh}"])
                return s

            def mk_mul(hh):
                def s():
                    nc.vector.tensor_mul(
                        OT_sb[64 * hh:64 * hh + DH, pa, W * sa:W * (sa + 1)],
                        osrc[0:DH, W * hh:W * (hh + 1)],
                        st[f"bcs{hh}"],
                    )
                    if hh == 1 and pa == NP - 1:
                        fq.extend(range(W * sa // 128, W * (sa + 1) // 128))
                return s

            def mk_mul_fine(hh, tl):
                def s():
                    nc.vector.tensor_mul(
                        OT_sb[64 * hh:64 * hh + DH, pa,
                              W * sa + 128 * tl:W * sa + 128 * (tl + 1)],
                        osrc[0:DH, W * hh + 128 * tl:W * hh + 128 * (tl + 1)],
                        st[f"bcs{hh}"][:, 128 * tl:128 * (tl + 1)],
                    )
                    if hh == 1:
                        fq.append(W * sa // 128 + tl)
                return s

            if fine:
                normq.extend([mk_rec(0), mk_bcs(0), mk_rec(1), mk_bcs(1)])
                for tl in range(W // 128):
                    normq.extend([mk_mul_fine(0, tl), mk_mul_fine(1, tl)])
            else:
                normq.extend([mk_rec(0), mk_bcs(0), mk_mul(0),
                              mk_rec(1), mk_bcs(1), mk_mul(1)])

        def emit_attnv():
            ga, sa, pa, ja = group(a[0])
            if ja == 0:
                avs[ga] = avP.tile([128, 2 * W], F32, tag="av", name="av")
            av = avs[ga]
            ex = exs.pop(a[0])
            for hh in range(2):
                nc.tensor.matmul(
                    av[0:DH + 1, W * hh:W * (hh + 1)],
                    V_sb[:, pa, ja, hh, :],
                    ex[:, W * hh:W * (hh + 1)],
                    start=(ja == 0),
                    stop=(ja == KT - 1),
                )
            if ja == KT - 1:
                av = avs.pop(ga)
                # two parallel copies (DVE: O rows fp16, ACT: Z row
                # fp32 for the reciprocal's bit-trick) release the
                # av accumulator for the next group; the norm chain
                # then runs from SBUF as dripped background stages
                oz = smallp.tile([DH, 2 * W], F16, tag="oz")
                nc.vector.tensor_copy(oz, av[0:DH, :])
                ozz = smallp.tile([1, 2 * W], F32, tag="ozz")
                nc.scalar.copy(ozz, av[DH:DH + 1, :])
                queue_norm(oz, ozz, 0, sa, pa, fine=(ga == NS * NP - 1))
            a[0] += 1

        a = [0]                       # attnV stream pointer
        with nc.named_scope("attn"):
            with tc.tile_pool(name="scP", bufs=2, space="PSUM") as scP:
                for i in range(SLOTS + LAG + 2):
                    # V-proj tasks, just-in-time for the attnV stream
                    vt = i - (LAG - 2)
                    if 0 <= vt < KT:
                        emit_vproj(vt)
                    # attnV stream: trails by LAG while the V projection
                    # drips in, then catches up to a 4-slot lag (2/slot)
                    # so the end-of-stream drain backlog is small. Runs
                    # BEFORE this slot's scores pair so the pair's wait on
                    # the exp(i-2) PSUM release overlaps attnV execution
                    # instead of idling the PE.
                    cap = 3 if i >= SLOTS else (2 if a[0] >= 2 * KT else 1)
                    done = 0
                    while (done < cap and a[0] < SLOTS
                           and a[0] <= i - (LAG if a[0] < 2 * KT else 4)):
                        emit_attnv()
                        done += 1
                    if normq:
                        normq.pop(0)()
                        if i >= SLOTS and normq:
                            normq.pop(0)()
                    elif fq and i % 2 == 1:
                        emit_fo(fq.pop(0), spP, "sp")
                    # scores pair + exp for slot i, emitted last
                    if i < SLOTS:
                        g, s, p, j = group(i)
                        sc = scP.tile([128, 2 * W], F32, tag="sc", name="sc")
                        # two row-tiled concurrent score matmuls
                        for hh in range(2):
                            nc.tensor.matmul(
                                sc[:, W * hh:W * (hh + 1)],
                                KT_sb[64 * hh:64 * (hh + 1), p,
                                      128 * j:128 * (j + 1)],
                                QT_sb[64 * hh:64 * (hh + 1), p,
                                      W * s:W * (s + 1)],
                                start=True, stop=True,
                            )
                        if DVE_EXP and j % 4 == 3:
                            exi = expp.tile([128, 2 * W], I16, tag="ex",
                                            name="exi")
                            nc.vector.tensor_scalar(
                                exi, sc, SCH_A, SCH_B,
                                mybir.AluOpType.mult,
                                mybir.AluOpType.add,
                            )
                            exs[i] = exi.bitcast(F16)
                        else:
                            ex = expp.tile([128, 2 * W], F16, tag="ex",
                                           name="ex")
                            nc.scalar.activation(
                                ex, sc, mybir.ActivationFunctionType.Exp,
                                scale=0.125,
                            )
                            exs[i] = ex
            # scP's 4 PSUM banks are dead now; give the tail's out-proj
            # chunks a 4-buffer pool so consecutive fo's don't serialize
            # on PSUM reuse
            with nc.named_scope("outproj"), \
                 tc.tile_pool(name="foP", bufs=4, space="PSUM") as foP:
                # interleave the remaining norm stages with out-proj
                # chunks so the PE drains fo while DVE/gpsimd normalize
                while normq or fq:
                    if normq:
                        normq.pop(0)()
                    if fq:
                        emit_fo(fq.pop(0), foP, "fo", tail=True)


def _get_nc():
    if "nc" not in _CACHE:
        _CACHE["nc"] = _build_nc()
    return _CACHE["nc"]


def make_in_maps(query, key, value, Wq, bq, Wk, bk, Wv, bv, Wo, bo):
    f32 = lambda a: np.asarray(a, dtype=np.float32)
    f16 = lambda a: np.ascontiguousarray(np.asarray(a, dtype=np.float32).astype(NPDT))
    query, key, value = f32(query), f32(key), f32(value)
    bq = np.ascontiguousarray(f32(bq))
    Wq, Wk, Wv, Wo = f32(Wq), f32(Wk), f32(Wv), f32(Wo)

    xT = [[f16(x[b].T) for b in range(B)] for x in (query, key, value)]
    in_maps = []
    for c in range(NCORES):
        b, g = c // GROUPS, c % GROUPS
        sl = slice(IS * g, IS * (g + 1))
        in_maps.append({
            "xqT": xT[0][b],
            "xkT": xT[1][b],
            "xvT": xT[2][b],
            "wq": f16(Wq[:, sl]),
            "wk": f16(Wk[:, sl]),
            "wv": f16(Wv[:, sl]),
            "wo": f16(Wo[sl, :]),
            "bq": np.ascontiguousarray(bq[sl]),
        })
    return in_maps


def combine_outputs(results, bv, Wo, bo):
    bo_eff = (
        np.asarray(bo, dtype=np.float32)
        + np.asarray(bv, dtype=np.float32) @ np.asarray(Wo, dtype=np.float32)
    )
    out = np.empty((B, S, D), dtype=np.float32)
    for b in range(B):
        acc = results[b * GROUPS]["out"].astype(np.float32)
        for g in range(1, GROUPS):
            acc = acc + results[b * GROUPS + g]["out"].astype(np.float32)
        out[b] = acc + bo_eff
    return out


def kernel(query, key, value, Wq, bq, Wk, bk, Wv, bv, Wo, bo):
    nc = _get_nc()
    in_maps = make_in_maps(query, key, value, Wq, bq, Wk, bk, Wv, bv, Wo, bo)
    try:
        res = run_bass_kernel_spmd(nc, in_maps, list(range(NCORES)))
    except Exception:
        # a fresh NEFF's first execution occasionally reports
        # NRT_EXEC_UNIT_UNRECOVERABLE; a retry reliably succeeds
        res = run_bass_kernel_spmd(nc, in_maps, list(range(NCORES)))
    return combine_outputs(res.results, bv, Wo, bo)
